# revision 16
# baseline (speedup 1.0000x reference)
"""Canny edge detector (nn_CannyNet) on 8 Trainium2 NeuronCores — v2.

Rebalanced rewrite of the v1 kernel for the TimelineSim/HW cost model where
GPSIMD(Pool) elementwise adds run at 0.42 efficiency (2127ns per 1024-col
f32 op vs DVE 1127 / Act 1038 / DVE-TSP 594).

Key changes vs v1:
- Fat 4-batch tiles: per-unit [122, 4*1026] tensors; one wide op replaces
  four per-plane ops (same elem cost, 4x fewer instructions/sems).
- NMS compares moved off Pool: direct DVE TT is_gt on f32 views.
- Masks fused via DVE scalar_tensor_tensor: mh = (sqx*(1+T1SQ) >= m2),
  mv = (sqx*(1+T3SQ) <= m2) — no T1SQ/T3SQ Act copies, no sqy buffer
  (sqy lives only inside m2 = Act Square(gy) + Pool add of sqx).
- Output: Act precomputes tht = Relu(m2 - THR2) per unit into THT (f16);
  each round's out = sel * tht is an all-f16 DVE TT (594ns vs the earlier
  1127ns mixed-dtype STT). Round outputs land in D1[u%%2] (slice b's mask
  is dead after that round's CPs); out-DMAs read D1, and the next
  same-parity D1 producer waits d_out. Host binarizes > 0.
- Engine split per unit (ns, ideal): Pool 66k (hgauss pA/pC/H + gx/ta/gy/
  d1/m2, all fat TT), DVE 78k (pB STT, masks, 8 fat compares, ANDs via
  half-fat product, 12 CP, 4 out STT, d1 relu TSP), Act 23k (h2, 8 psum
  evacs, sqx, sqy->m2), PE 16 fp32r matmuls ~30k.
- Buffer reuse: p1t/GX share one [128,4096]; TA lives in TCX (offset 1);
  GY lives in TCY (offset 1); SQX overwrites TA in TCX; SQY is written
  by Act Square directly into M2; ANDs collapse in place into CSA/CSB.
  M2P/M2M single-buffered (shift DMAs via Pool SWDGE right after M2add,
  out DMAs on the sync queue lagged two units -- both orderings matter:
  earlier placements serialized the production/NMS phases).
- Unit 3 is not a dummy: it is a half-width (512-col) strip of one of the
  three leftover k=8 units (core i takes strip i%2 of channel i//2; cores
  6,7 duplicate a strip, host ignores). Strip geometry: x 520 cols,
  tcx/H width 516 (_wh), m2-center 514 (_wc) at block offset 2 (_mlo), out
  512. Strip halo m2/tcx cols are computed exactly (no pad reads); the
  image-edge side is zeroed via the host 'hmask' input to reproduce the
  reference's between-stage zero-pad truncation (blur pad for gx at cols
  0/1023, gm pad for the NMS neighbor).
- Unit 0's fat chain (GX/TA/GY/D1/M2add) runs on DVE instead of Pool:
  DVE is idle during pipeline fill, and removing 41us from Pool's serial
  stream shifts every later unit's M2add (the NMS gate) earlier. Its
  shift DMAs moved to the Act HWDGE queue.
- Fill trims: prologue memsets deferred after planes(0); x-DMA WAR uses
  a g_pc sem (pC is the last Pool x reader, not Hadd); unit-0 x DMAs
  split across the sync and Act queues.
TimelineSim: 376.1us/core vs 615us for the v1 kernel (1.63x); 16 flips
(rel 2.4e-3) on PJRT hardware, same as v1. Structure of the remaining
time: DVE is the bound (busy 279us / 74.1%): first DVE op ~20us, fat0
starts ~70us (gated by PE(0) pstate-ramped matmuls + Hadd(1)), then DVE
runs nearly gap-free to the end; the tail (~70us of nms(2)/masks(3)/
nms(3)) is pure DVE with Pool idle. Further gains need DVE work
reduction: the f32 compare + copy_predicated NMS block (~50us/unit) is
the floor; f16 compares would halve it but measured ~1200 flips vs the
~1066 budget. Pool-assisted AND/sub offload in the tail nets ~0 after
semaphore round-trips (Pool fat sub 8.2us vs DVE 4.5us consumption).
PE pstate note: fp32 matmuls cost 2429/1707/853ns (low/mid/full) per
512 cols; full speed needs >3us of CONTINUOUS PE execution and any
idle resets the streak (instruction_cost_v2.rs ~line 862). A dummy-MM
warmup is blocked: PSUM is exactly full (pcx/pcy x2 = 8 banks), so
dummies have no safe write target, and streaks still break at the
per-plane a_ev/g_H waits. Freeing a PSUM bank (e.g. 512-col matmul
tiles with 3-way ping-pong) would enable it and save ~15-25us.
SBUF note (measured): exactly 7744B/partition free (alloc of a 16416B
probe tensor inside build_nc fails with "need 16416B, have 7744B";
base=221600, top=229344). The TCX-double-buffer lever (+16.4KB, ~9us)
is confirmed BLOCKED; only sub-7.7KB scratch is available. (Allocating
probes AFTER build_nc returns is meaningless -- its ExitStack has
already freed the tensors.)
AP note: stride-0 broadcast operands (ap.unsqueeze(1).broadcast_to())
ARE legal on DVE TT on HW (verified via PJRT). Not exploitable for the
NMS compare merges though: each round's pv/mv pair spans two different
tensors (M2P/M2M), and same-tensor pairs need stride -2 or -1 outer
dims that no AP constructor (rearrange/broadcast_to) can express.
"""
import math
import os
import numpy as np

import concourse.bass as bass
import concourse.mybir as mybir
from concourse.bass_utils import run_bass_kernel_spmd

ALU = mybir.AluOpType
AF = mybir.ActivationFunctionType
DT = mybir.dt.float32
F16 = mybir.dt.float16
U16 = mybir.dt.uint16

B, C, H_IMG, W = 4, 3, 1024, 1024
NU = 4            # units per core
M = 122           # m2/out row span per unit (out 120 + 2)
XR = 128          # x-tile rows
FW = 1028         # x-tile width (cols -2..1025)
MW = 1026         # per-batch m2 block width (cols -1..1024)
W4 = 4 * W        # fat width without pads
MW4 = 4 * MW      # fat width with pads
FW4 = 4 * FW

_g = np.exp(-0.5 * np.arange(-2, 3, dtype=np.float64) ** 2)
G1 = _g[1]
R0 = float(np.float32(_g[0] / _g[1]))   # g0/g1
R2 = float(np.float32(1.0 / _g[1]))     # 1/g1
C0 = float(np.float32(2.0 + 2.0 * R0 + R2))  # +1 fold (pads are -1 -> 0)
THR2 = float(np.float32((400.0 / (127.5 * G1)) ** 2))
_t1 = math.tan(22.5 * 3.14159 / 180.0)
_t3 = math.tan(67.5 * 3.14159 / 180.0)
INVR0 = float(np.float32(1.0 / R0))
R2R0 = float(np.float32(R2 / R0))
C0R0 = float(np.float32(C0 / R0))
T1SQ1 = float(np.float32(1.0 + _t1 * _t1))  # masks vs m2: sqx*(1+t^2) vs m2
T3SQ1 = float(np.float32(1.0 + _t3 * _t3))

# units and core assignment (identical to v1)
UNITS = [(c, k) for k in range(9) for c in range(3)]  # 27
CORE_UNITS = []
for i in range(8):
    us = [UNITS[i], UNITS[i + 8], UNITS[i + 16]]
    us.append(UNITS[24 + i] if i < 3 else UNITS[i])  # dummy repeat for cores 3..7
    CORE_UNITS.append(us)


def _unit_rows(k):
    if k < 8:
        return 120 * k - 4, 120 * k
    return 900, 960


def _make_bands():
    n = H_IMG
    G = np.zeros((n, n), np.float64)
    for kk in range(-2, 3):
        v = _g[kk + 2]
        for o in range(max(0, -kk), min(n, n - kk)):
            G[o, o + kk] = v
    S121 = np.zeros((n, n), np.float64)
    S101 = np.zeros((n, n), np.float64)
    for o in range(n):
        for kk, w1, w2 in ((-1, 1.0, 1.0), (0, 2.0, 0.0), (1, 1.0, -1.0)):
            i = o + kk
            if 0 <= i < n:
                S121[o, i] = w1
                if kk != 0:
                    S101[o, i] = w2
    CX = (S121 @ G).astype(np.float32)
    CY = (S101 @ G).astype(np.float32)
    return CX, CY


def _band_lhsT(Cm, k):
    xbase, out0 = _unit_rows(k)
    out = np.zeros((XR, M), np.float32)
    for m in range(M):
        orow = out0 - 1 + m
        if not (0 <= orow < H_IMG):
            continue
        for d in range(-3, 4):
            irow = orow + d
            kr = irow - xbase
            if 0 <= irow < H_IMG and 0 <= kr < XR:
                out[kr, m] = Cm[orow, irow]
    return out


def _fat(ap, blk, lo, hi):
    """3-level view: [P, 4, hi-lo] over blocks of width blk."""
    return ap.rearrange("p (q w) -> p q w", q=4)[:, :, lo:hi]


def _wc(u):
    """working (m2-center) width per batch block: full 1024, strip 514."""
    return 1024 if u < 3 else 514


def _wout(u):
    return 1024 if u < 3 else 512


def _mlo(u):
    """block offset of the m2-center: full 1, strip 2 (tcx spans [1:517])."""
    return 1 if u < 3 else 2


def _wh(u):
    """tcx/H working width: full 1024, strip 516 (m2-center + 1 each side)."""
    return 1024 if u < 3 else 516


def build_nc():
    nc = bass.Bass()
    _ct = nc.alloc_sbuf_tensor("const-float32-negthr2", [128, 1], DT)
    nc.gpsimd.memset(_ct.ap(), -THR2)
    nc.const_aps.aps[(mybir.dt.float32, -THR2)] = _ct.ap()
    nc.all_engine_barrier()
    xin = nc.declare_dram_parameter("xin", [NU, B, XR, FW], DT, isOutput=False)
    bands = nc.declare_dram_parameter("bands", [128, NU * 2 * M], DT,
                                      isOutput=False)
    outd = nc.declare_dram_parameter("out", [NU, B, 120, W], F16, isOutput=True)
    hmaskd = nc.declare_dram_parameter("hmask", [M, 8], DT, isOutput=False)

    from contextlib import ExitStack
    es = ExitStack()
    ent = es.enter_context

    # ---- SBUF ----
    xF = ent(nc.sbuf_tensor("xF", [XR, FW4], DT))
    HF = ent(nc.sbuf_tensor("HF", [XR, W4], DT))
    PG = ent(nc.sbuf_tensor("PG", [XR, W4], DT))      # p1t (hgauss) / GX (fat)
    TCX = ent(nc.sbuf_tensor("TCX", [M, MW4], DT))
    TCY = ent(nc.sbuf_tensor("TCY", [M, MW4], DT))
    M2 = [ent(nc.sbuf_tensor(f"M2_{p}", [M, MW4], DT)) for p in range(2)]
    M2P = ent(nc.sbuf_tensor("M2P", [M, MW4], DT))
    M2M = ent(nc.sbuf_tensor("M2M", [M, MW4], DT))
    D1 = [ent(nc.sbuf_tensor(f"D1_{p}", [M, W4], F16)) for p in range(2)]
    MH = ent(nc.sbuf_tensor("MH", [M, W4], F16))
    MV = ent(nc.sbuf_tensor("MV", [M, W4], F16))
    CSA = ent(nc.sbuf_tensor("CSA", [M, W4], F16))
    CSB = ent(nc.sbuf_tensor("CSB", [M, W4], F16))
    THT = ent(nc.sbuf_tensor("THT", [M, W4], F16))
    bnd = ent(nc.sbuf_tensor("bnd", [128, NU * 2 * M], DT))
    hmask = ent(nc.sbuf_tensor("hmask_s", [M, 8], DT))
    pcx = [ent(nc.psum_tensor(f"pcx{j}", [M, W], DT)) for j in range(2)]
    pcy = [ent(nc.psum_tensor(f"pcy{j}", [M, W], DT)) for j in range(2)]

    GX = PG[0:M, :]  # [122, 4096] alias of p1t rows

    # ---- semaphores ----
    d_b = ent(nc.semaphore("d_b"))
    d_x = [ent(nc.semaphore(f"d_x{b}")) for b in range(B)]
    d_sh = ent(nc.semaphore("d_sh"))
    d_out = ent(nc.semaphore("d_out"))
    a_h2 = ent(nc.semaphore("a_h2"))    # Act h2 fat done (per unit)
    a_ev = ent(nc.semaphore("a_ev"))    # Act evac done (2 per plane)
    a_sq = ent(nc.semaphore("a_sq"))
    a_d1 = ent(nc.semaphore("a_d1"))
    a_tht = ent(nc.semaphore("a_tht"))    # Act SQX (2u+1), SQY->M2 (2u+2)
    g_pa = ent(nc.semaphore("g_pa"))    # Pool pA fat done (per unit)
    g_pc = ent(nc.semaphore("g_pc"))    # Pool pC done (last Pool x reader)
    v_pb = ent(nc.semaphore("v_pb"))    # DVE pB fat done (per unit)
    g_H = ent(nc.semaphore("g_H"))      # Pool Hadd fat done (per unit)
    g_f = ent(nc.semaphore("g_f"))      # Pool fat ops: 5u+{1..5}=GX,TA,GY,D1,M2
    v_mk = ent(nc.semaphore("v_mk"))    # DVE masks done (2 per unit)
    v_nms = ent(nc.semaphore("v_nms"))  # DVE round out done (4u+b+1)
    pe = ent(nc.semaphore("pe"))        # PE per-plane matmul groups done
    block = ent(nc.Block())

    # fat views over M2-layout tensors
    def mv_(t, lo, hi):
        return _fat(t[:], MW, lo, hi)

    def CTRu(p, u):
        lo = _mlo(u)
        return mv_(M2[p], lo, lo + _wc(u))

    # per-round (pv, mv) neighbor views
    def dirs(p, b, u):
        wc = _wc(u)
        lo = _mlo(u)
        if b == 0:
            return mv_(M2[p], lo + 1, lo + 1 + wc), mv_(M2[p], lo - 1, lo - 1 + wc)
        if b == 1:
            return mv_(M2P, lo + 1, lo + 1 + wc), mv_(M2M, lo - 1, lo - 1 + wc)
        if b == 2:
            return mv_(M2P, lo, lo + wc), mv_(M2M, lo, lo + wc)
        return mv_(M2P, lo - 1, lo - 1 + wc), mv_(M2M, lo + 1, lo + 1 + wc)

    def xv(lo, hi):
        return _fat(xF[:], FW, lo, hi)

    HFr = _fat(HF[:], W, 0, 1024)
    PGr = _fat(PG[:], W, 0, 1024)
    GXr = _fat(GX, W, 0, 1024)

    @block.sync
    def _(sync):
        def outs(u):
            wo = _wout(u)
            so = (_mlo(u) + 1 if u == 3 else _mlo(u)) - _mlo(u)
            for b in range(B):
                sync.wait_ge(v_nms, 4 * u + b + 1)
                sync.dma_start(out=outd[u, b][:, 0:wo],
                               in_=D1[u % 2][1:121, b * W + so:b * W + so + wo]
                               ).then_inc(d_out, 16)

        sync.dma_start(out=bnd[:], in_=bands[:]).then_inc(d_b, 16)
        sync.dma_start(out=hmask[:], in_=hmaskd[:]).then_inc(d_b, 16)
        for u in range(NU):
            for b in range(B):
                if u == 0 and b >= 2:
                    continue  # issued from the Act queue to halve fill latency
                if u >= 1:
                    # x consumers of unit u-1: Act h2, DVE pB, Pool pC
                    sync.wait_ge(a_h2, u)
                    sync.wait_ge(v_pb, u)
                    sync.wait_ge(g_pc, u)
                sync.dma_start(out=xF[:, b * FW:(b + 1) * FW],
                               in_=xin[u, b]).then_inc(d_x[b], 16)
            if u >= 2:
                outs(u - 2)
        outs(NU - 2)
        outs(NU - 1)
        sync.wait_ge(d_out, 16 * NU * B)

    @block.scalar
    def _(act):
        def squares(u):
            wc = _wc(u)
            lo = _mlo(u)
            # SQX = Square(GX) -> TCX[.., lo:lo+wc]; waits GY done (TA dead)
            act.wait_ge(g_f, 5 * u + 3)
            nc.scalar.activation(out=_fat(TCX[:], MW, lo, lo + wc),
                                 in_=_fat(GX, W, 0, wc),
                                 func=AF.Square).then_inc(a_sq, 1)
            # SQY = Square(GY-in-TCY) -> M2 center; M2 WAR vs NMS(u-2)
            if u >= 2:
                act.wait_ge(v_nms, 4 * (u - 1))
            nc.scalar.activation(out=mv_(M2[u % 2], lo, lo + wc),
                                 in_=_fat(TCY[:], MW, lo, lo + wc),
                                 func=AF.Square).then_inc(a_sq, 1)
            # d1 relu in place (mask: +0 stays false, negatives clamp to +0)
            act.wait_ge(g_f, 5 * u + 4)
            nc.scalar.activation(out=_fat(D1[u % 2][:], W, 0, wc),
                                 in_=_fat(D1[u % 2][:], W, 0, wc),
                                 func=AF.Relu).then_inc(a_d1, 1)
            # tht = Relu(m2 - THR2) per unit (was a per-round DVE STT)
            act.wait_ge(g_f, 5 * u + 5)
            if u >= 1:
                act.wait_ge(v_nms, 4 * u)  # THT WAR vs OUT(u-1) reads
            nc.scalar.activation(out=_fat(THT[:], W, 0, wc),
                                 in_=mv_(M2[u % 2], lo, lo + wc),
                                 func=AF.Relu, bias=-THR2).then_inc(a_tht, 1)

        for b in (2, 3):
            act.dma_start(out=xF[:, b * FW:(b + 1) * FW],
                          in_=xin[0, b]).then_inc(d_x[b], 16)
        for u in range(NU):
            # h2 fat: HF = x[:, 2:1026]*R2R0 + C0R0
            for b in range(B):
                act.wait_ge(d_x[b], 16 * (u + 1))
            if u >= 1:
                act.wait_ge(pe, 4 * u)  # HF WAR vs PE(u-1)
            wh = _wh(u)
            nc.scalar.activation(out=_fat(HF[:], W, 0, wh), in_=xv(2, 2 + wh),
                                 func=AF.Copy,
                                 scale=R2R0, bias=C0R0).then_inc(a_h2, 1)
            if u >= 1:
                squares(u - 1)
            if u == 1:
                # u=0 shift DMAs (fat(0) runs on DVE; its SWDGE slot is gone)
                act.wait_ge(g_f, 5)
                act.dma_start(out=M2P[0:M - 1, :],
                              in_=M2[0][1:M, :]).then_inc(d_sh, 16)
                act.dma_start(out=M2M[1:M, :],
                              in_=M2[0][0:M - 1, :]).then_inc(d_sh, 16)
            # evacs per plane
            for b in range(B):
                n = 4 * u + b
                act.wait_ge(pe, n + 1)
                if u >= 1 and b == 0:
                    # TCX/TCY WAR vs masks(u-1) + Pool fat(u-1)
                    act.wait_ge(v_mk, 2 * u)
                    act.wait_ge(g_f, 5 * u)
                wh = _wh(u)
                nc.scalar.activation(out=TCX[:, b * MW + 1:b * MW + 1 + wh],
                                     in_=pcx[n % 2][:, 0:wh],
                                     func=AF.Copy).then_inc(a_ev, 1)
                nc.scalar.activation(out=TCY[:, b * MW + 1:b * MW + 1 + wh],
                                     in_=pcy[n % 2][:, 0:wh],
                                     func=AF.Copy).then_inc(a_ev, 1)
        squares(NU - 1)

    @block.gpsimd
    def _(g):
        def prologue_memsets():
            # pad cols of TCX/TCY/M2 and shift-pad rows; emitted after
            # planes(0) so they don't delay the first production chain
            # (needed before: evac(0) pads -> GX(0)/compares, shift DMAs)
            for q in range(4):
                for t in (TCX, TCY):
                    g.memset(t[:, q * MW:q * MW + 1], 0.0)
                    g.memset(t[:, q * MW + 1025:q * MW + 1026], 0.0)
                for p in range(2):
                    g.memset(M2[p][:, q * MW:q * MW + 1], 0.0)
                    g.memset(M2[p][:, q * MW + 1025:q * MW + 1026], 0.0)
            g.memset(M2P[96:M, :], 0.0)
            g.memset(M2M[0:32, :], 0.0)

        def planes(u):
            # pA = x[1:1025] + x[3:1027] -> PG
            for b in range(B):
                g.wait_ge(d_x[b], 16 * (u + 1))
            if u >= 2:
                g.wait_ge(a_sq, 2 * (u - 2) + 1)  # PG WAR vs SQX(u-2)
                g.wait_ge(g_f, 5 * (u - 2) + 4)   # PG WAR vs D1(u-2)
            wh = _wh(u)
            PGw = _fat(PG[:], W, 0, wh)
            HFw = _fat(HF[:], W, 0, wh)
            g.tensor_tensor(out=PGw, in0=xv(1, 1 + wh), in1=xv(3, 3 + wh),
                            op=ALU.add).then_inc(g_pa, 1)
            # pC = pB + x[4:4+wh] (pB from DVE, in PG)
            g.wait_ge(v_pb, u + 1)
            g.tensor_tensor(out=PGw, in0=PGw, in1=xv(4, 4 + wh),
                            op=ALU.add).then_inc(g_pc, 1)
            # Hadd: HF += PG
            g.wait_ge(a_h2, u + 1)
            g.tensor_tensor(out=HFw, in0=HFw, in1=PGw,
                            op=ALU.add).then_inc(g_H, 1)

        def fat(u):
            wc = _wc(u)
            lo = _mlo(u)
            # re-zero TCX pad col 1025 (TA(u-1) overwrote it); full units only
            if 1 <= u <= 2:
                g.memset(_fat(TCX[:], MW, 1025, 1026), 0.0)
            g.wait_ge(a_ev, 8 * (u + 1))
            if u == 3:
                # zero the beyond-image-edge tcx/tcy col (blur zero-pad
                # truncation); interior strip-boundary cols stay computed
                for t in (TCX, TCY):
                    g.tensor_tensor(out=_fat(t[:], MW, lo, lo + 1),
                                    in0=_fat(t[:], MW, lo, lo + 1),
                                    in1=hmask[:, 0:4].rearrange("p (q w) -> p q w", q=4),
                                    op=ALU.mult)
                    g.tensor_tensor(out=_fat(t[:], MW, lo + wc - 1, lo + wc),
                                    in0=_fat(t[:], MW, lo + wc - 1, lo + wc),
                                    in1=hmask[:, 4:8].rearrange("p (q w) -> p q w", q=4),
                                    op=ALU.mult)
            if u >= 1:
                g.wait_ge(a_sq, 2 * (u - 1) + 1)  # GX WAR vs SQX(u-1)
            g.tensor_tensor(out=_fat(GX, W, 0, wc),
                            in0=_fat(TCX[:], MW, lo - 1, lo - 1 + wc),
                            in1=_fat(TCX[:], MW, lo + 1, lo + 1 + wc),
                            op=ALU.subtract).then_inc(g_f, 1)
            # TA -> TCX[.., lo:lo+wc+1] = TCY[.., lo-1:lo+wc] + TCY[.., lo:lo+wc+1]
            if u >= 1:
                g.wait_ge(v_mk, 2 * u)  # TCX WAR vs masks(u-1) reading SQX
            g.tensor_tensor(out=_fat(TCX[:], MW, lo, lo + wc + 1),
                            in0=_fat(TCY[:], MW, lo - 1, lo + wc),
                            in1=_fat(TCY[:], MW, lo, lo + wc + 1),
                            op=ALU.add).then_inc(g_f, 1)
            # GY -> TCY[.., lo:lo+wc] = TA[.., lo:lo+wc] + TA[.., lo+1:lo+1+wc]
            if u >= 1:
                g.wait_ge(a_sq, 2 * (u - 1) + 2)  # TCY WAR vs SQY(u-1)
            g.tensor_tensor(out=_fat(TCY[:], MW, lo, lo + wc),
                            in0=_fat(TCX[:], MW, lo, lo + wc),
                            in1=_fat(TCX[:], MW, lo + 1, lo + 1 + wc),
                            op=ALU.add).then_inc(g_f, 1)
            # D1 = GX * GY -> f16; parity WAR vs CP(u-2) and outs(u-2)
            if u >= 2:
                g.wait_ge(v_nms, 4 * (u - 1))
                g.wait_ge(d_out, 64 * (u - 1))
            g.tensor_tensor(out=_fat(D1[u % 2][:], W, 0, wc),
                            in0=_fat(GX, W, 0, wc),
                            in1=_fat(TCY[:], MW, lo, lo + wc),
                            op=ALU.mult).then_inc(g_f, 1)
            # M2 += SQX (in TCX)
            g.wait_ge(a_sq, 2 * u + 2)
            if u == 3:
                g.tensor_tensor(out=mv_(M2[u % 2], lo, lo + wc),
                                in0=mv_(M2[u % 2], lo, lo + wc),
                                in1=_fat(TCX[:], MW, lo, lo + wc),
                                op=ALU.add)
                # zero edge-side halo cols via host mask (interior stays)
                g.tensor_tensor(out=mv_(M2[u % 2], lo, lo + 1),
                                in0=mv_(M2[u % 2], lo, lo + 1),
                                in1=hmask[:, 0:4].rearrange("p (q w) -> p q w", q=4),
                                op=ALU.mult)
                g.tensor_tensor(out=mv_(M2[u % 2], lo + wc - 1, lo + wc),
                                in0=mv_(M2[u % 2], lo + wc - 1, lo + wc),
                                in1=hmask[:, 4:8].rearrange("p (q w) -> p q w", q=4),
                                op=ALU.mult).then_inc(g_f, 1)
            else:
                g.tensor_tensor(out=mv_(M2[u % 2], lo, lo + wc),
                                in0=mv_(M2[u % 2], lo, lo + wc),
                                in1=_fat(TCX[:], MW, lo, lo + wc),
                                op=ALU.add).then_inc(g_f, 1)
            # shift DMAs via SWDGE (in-order after M2add; WAR vs NMS(u-1))
            if u >= 1:
                g.wait_ge(v_nms, 4 * u)
            g.dma_start(out=M2P[0:M - 1, :],
                        in_=M2[u % 2][1:M, :]).then_inc(d_sh, 16)
            g.dma_start(out=M2M[1:M, :],
                        in_=M2[u % 2][0:M - 1, :]).then_inc(d_sh, 16)

        planes(0)
        prologue_memsets()
        planes(1)
        planes(2)
        fat(1)
        planes(3)
        fat(2)
        fat(3)

    @block.vector
    def _(v):
        def pb(u):
            # pB = pA * INVR0 + x[0:wh] (strip unit runs narrow)
            wh = _wh(u)
            v.wait_ge(g_pa, u + 1)
            for b in range(B):
                v.wait_ge(d_x[b], 16 * (u + 1))
            v.scalar_tensor_tensor(out=_fat(PG[:], W, 0, wh),
                                   in0=_fat(PG[:], W, 0, wh), scalar=INVR0,
                                   in1=xv(0, wh), op0=ALU.mult,
                                   op1=ALU.add).then_inc(v_pb, 1)

        def masks(u):
            wc = _wc(u)
            lo = _mlo(u)
            v.wait_ge(g_f, 5 * u + 5)
            sqx = _fat(TCX[:], MW, lo, lo + wc)
            ctr = CTRu(u % 2, u)
            v.scalar_tensor_tensor(out=_fat(MH[:], W, 0, wc),
                                   in0=sqx, scalar=T1SQ1, in1=ctr,
                                   op0=ALU.mult, op1=ALU.is_ge).then_inc(v_mk, 1)
            v.scalar_tensor_tensor(out=_fat(MV[:], W, 0, wc),
                                   in0=sqx, scalar=T3SQ1, in1=ctr,
                                   op0=ALU.mult, op1=ALU.is_le).then_inc(v_mk, 1)


        def nms_round(u, b):
            p = u % 2
            wc = _wc(u)
            wo = _wout(u)
            olo = _mlo(u) if u < 3 else _mlo(u) + 1
            pv, mvv = dirs(p, b, u)
            if b == 0:
                v.wait_ge(g_f, 5 * u + 5)
            else:
                v.wait_ge(d_sh, 32 * (u + 1))
            csa4 = CSA[:].rearrange("p (q w) -> p q w", q=4)
            csb4 = CSB[:].rearrange("p (q w) -> p q w", q=4)
            v.tensor_tensor(out=csa4[:, :, 0:wc], in0=CTRu(p, u), in1=pv,
                            op=ALU.is_gt)
            v.tensor_tensor(out=csb4[:, :, 0:wc], in0=CTRu(p, u), in1=mvv,
                            op=ALU.is_gt)
            # ANDs: [amh|amv] in CSA blocks 0-1, [ad1|a135] in CSB blocks 0-1
            v.tensor_tensor(out=csa4[:, 0:2, 0:wc], in0=csa4[:, 0:2, 0:wc],
                            in1=csa4[:, 2:4, 0:wc], op=ALU.mult)
            v.tensor_tensor(out=csb4[:, 0:2, 0:wc], in0=csb4[:, 0:2, 0:wc],
                            in1=csb4[:, 2:4, 0:wc], op=ALU.mult)
            # mux into CSB block 1 (= a135 slice)
            sel = CSB[:, W:W + wc]
            v.wait_ge(a_d1, u + 1)
            v.copy_predicated(out=sel,
                              mask=D1[p].bitcast(U16)[:, b * W:b * W + wc],
                              data=CSB[:, 0:wc])
            v.copy_predicated(out=sel, mask=MV.bitcast(U16)[:, b * W:b * W + wc],
                              data=CSA[:, W:W + wc])
            v.copy_predicated(out=sel, mask=MH.bitcast(U16)[:, b * W:b * W + wc],
                              data=CSA[:, 0:wc])
            # out = sel * tht -> f16 into D1[p] (slice b's mask is dead
            # after this round's CPs; DMA reads it before D1(u+2) writes)
            v.wait_ge(a_tht, u + 1)
            so = olo - _mlo(u)
            v.tensor_tensor(out=D1[p][:, b * W + so:b * W + so + wo],
                            in0=CSB[:, W + so:W + so + wo],
                            in1=THT[:, b * W + so:b * W + so + wo],
                            op=ALU.mult).then_inc(v_nms, 1)

        pb(0)
        pb(1)
        # unit 0 fat chain on DVE (shortens Pool's serial path by ~41us;
        # DVE is idle during fill anyway). PG is shared with Pool planes(1)
        # scratch -> wait Hadd(1) before writing GX.
        u0 = 0
        wc0 = _wc(u0)
        lo0 = _mlo(u0)
        v.wait_ge(a_ev, 8)
        v.wait_ge(g_H, 2)  # PG WAR vs Pool planes(1) pA/pC
        v.tensor_tensor(out=_fat(GX, W, 0, wc0),
                        in0=_fat(TCX[:], MW, lo0 - 1, lo0 - 1 + wc0),
                        in1=_fat(TCX[:], MW, lo0 + 1, lo0 + 1 + wc0),
                        op=ALU.subtract).then_inc(g_f, 1)
        v.tensor_tensor(out=_fat(TCX[:], MW, lo0, lo0 + wc0 + 1),
                        in0=_fat(TCY[:], MW, lo0 - 1, lo0 + wc0),
                        in1=_fat(TCY[:], MW, lo0, lo0 + wc0 + 1),
                        op=ALU.add).then_inc(g_f, 1)
        v.tensor_tensor(out=_fat(TCY[:], MW, lo0, lo0 + wc0),
                        in0=_fat(TCX[:], MW, lo0, lo0 + wc0),
                        in1=_fat(TCX[:], MW, lo0 + 1, lo0 + 1 + wc0),
                        op=ALU.add).then_inc(g_f, 1)
        v.tensor_tensor(out=_fat(D1[0][:], W, 0, wc0),
                        in0=_fat(GX, W, 0, wc0),
                        in1=_fat(TCY[:], MW, lo0, lo0 + wc0),
                        op=ALU.mult).then_inc(g_f, 1)
        v.wait_ge(a_sq, 2)
        v.tensor_tensor(out=mv_(M2[0], lo0, lo0 + wc0),
                        in0=mv_(M2[0], lo0, lo0 + wc0),
                        in1=_fat(TCX[:], MW, lo0, lo0 + wc0),
                        op=ALU.add).then_inc(g_f, 1)
        masks(0)
        pb(2)
        for b in range(B):
            nms_round(0, b)
        masks(1)
        pb(3)
        for b in range(B):
            nms_round(1, b)
        masks(2)
        for b in range(B):
            nms_round(2, b)
        masks(3)
        for b in range(B):
            nms_round(3, b)

    @block.tensor
    def _(t):
        t.wait_ge(d_b, 32)
        for u in range(NU):
            bx = bnd[:, (u * 2 + 0) * M:(u * 2 + 1) * M]
            by = bnd[:, (u * 2 + 1) * M:(u * 2 + 2) * M]
            t.wait_ge(g_H, u + 1)
            cuts = [(0, 512), (512, 1024)] if u < 3 else [(0, 512), (512, 516)]
            for b in range(B):
                n = 4 * u + b
                if n >= 2:
                    t.wait_ge(a_ev, 2 * (n - 2) + 2)
                p, q = pcx[n % 2], pcy[n % 2]
                hh = HF[:, b * W:(b + 1) * W]
                for ci, (lo, hi) in enumerate(cuts):
                    nc.tensor.matmul(p[:, lo:hi], bx, hh[:, lo:hi],
                                     start=True, stop=True)
                    mm = nc.tensor.matmul(q[:, lo:hi], by, hh[:, lo:hi],
                                          start=True, stop=True)
                    if ci == len(cuts) - 1:
                        mm.then_inc(pe, 1)

    es.close()
    return nc


_NC_CACHE = {}
LAST_RESULT = None


def kernel(img, gauss_h=None, gauss_v=None, sobel_h=None, sobel_v=None,
           dir_w=None, **_):
    img = np.asarray(img, dtype=np.float32)
    assert img.shape == (B, C, H_IMG, W)

    pad = np.full((B, C, H_IMG + 8, W + 8), -1.0, np.float32)
    pad[:, :, 4:4 + H_IMG, 4:4 + W] = img

    CX, CY = _make_bands()
    band_cache = {}
    for c, k in UNITS:
        if k not in band_cache:
            band_cache[k] = (_band_lhsT(CX, k), _band_lhsT(CY, k))

    in_maps = []
    for i in range(8):
        xin = np.full((NU, B, XR, FW), -1.0, np.float32)
        bands = np.zeros((128, NU * 2 * M), np.float32)
        for u in range(3):
            c, k = CORE_UNITS[i][u]
            xbase, _o = _unit_rows(k)
            r = xbase + 4
            for b in range(B):
                xin[u, b] = pad[b, c, r:r + XR, 2:2 + FW]
            bx, by = band_cache[k]
            bands[:, (u * 2) * M:(u * 2 + 1) * M] = bx * np.float32(R0)
            bands[:, (u * 2 + 1) * M:(u * 2 + 2) * M] = by * np.float32(R0)
        # u=3: half-width strip of the k=8 leftover units (c = i//2, h = i%2;
        # cores 6,7 run a duplicate strip whose output is ignored)
        si = min(i, 5)
        c_s, h_s = si // 2, si % 2
        r = 900 + 4
        col = 512 * h_s  # padded-coords of image col 512h-4
        for b in range(B):
            xin[3, b][:, 0:520] = pad[b, c_s, r:r + XR, col:col + 520]
        bx, by = band_cache[8]
        bands[:, 6 * M:7 * M] = bx * np.float32(R0)
        bands[:, 7 * M:8 * M] = by * np.float32(R0)
        hmask = np.ones((M, 8), np.float32)
        if h_s == 0:
            hmask[:, 0:4] = 0.0  # left halo is the image edge -> zero
        else:
            hmask[:, 4:8] = 0.0  # right halo is the image edge -> zero
        in_maps.append({"xin": xin, "bands": bands, "hmask": hmask})

    key = "nc"
    if key not in _NC_CACHE:
        _NC_CACHE[key] = build_nc()
    nc = _NC_CACHE[key]
    kw = {}
    if os.environ.get('KTRACE'):
        kw = dict(trace=True,
                  trace_cores=[int(x) for x in
                               os.environ.get('KTRACE_CORES', '0').split(',')])
    r = run_bass_kernel_spmd(nc, in_maps, list(range(8)), **kw)
    global LAST_RESULT
    LAST_RESULT = r
    res = r.results

    out = np.zeros((B, C, H_IMG, W), np.float32)
    for i in range(8):
        for u in range(3):
            c, k = CORE_UNITS[i][u]
            _xb, out0 = _unit_rows(k)
            out[:, c, out0:out0 + 120, :] = (
                res[i]["out"][u, :, :120, :] > 0).astype(np.float32)
        if i < 6:
            c_s, h_s = i // 2, i % 2
            out[:, c_s, 960:1024, 512 * h_s:512 * h_s + 512] = (
                res[i]["out"][3, :, :64, 0:512] > 0).astype(np.float32)
    mn, mx = out.min(), out.max()
    return ((out - mn) / (mx - mn)).astype(np.float32)
